# revision 24
# baseline (speedup 1.0000x reference)
"""Causal self-attention (B=1, T=2048, C=1024, H=16, RoPE) on 8 TRN2 NeuronCores.

Sharding: 2 heads per core (tensor parallel on w_qkv columns / w_proj rows).
Each core computes a full-shape partial output in bf16; the host sums the 8
partials (the tensor-parallel all-reduce, done at gather time).

Per-core pipeline (v2, all-bf16 datapath):
  - qkv = x @ w_local in bf16 (lhsT = xT chunk tiles shipped bf16; per-chunk
    tiles keep tile-granular dependencies narrow)
  - RoPE on q,k (DVE muls + gpsimd combines), output bf16; q,k PE-transposed
    to (dim, T) per chunk
  - scores S^T[j, i] per key-tile in bf16 (contraction D=64); exp on Act with
    a -2 bias (cancels in softmax); causal masking via gpsimd affine_select
    restricted to the 128-wide diagonal block
  - PV transposed: lhsT = exp tile (queries become out partitions), rhs = v
    bf16 [128, 65] with a ones column, so out[q, s, 64] is the softmax
    denominator per query partition - normalize with a [128,2,1] reciprocal
    broadcast along free (no partition_broadcast needed); fully-masked
    slice/key-tile combinations are skipped
  - normalized attention transposed back (PE) for the bf16 projection;
    out partials written bf16, one DMA per chunk half
  - scheduling: every chunk's S runs one pair ahead of PV; the previous
    chunk's normalize + projection and the next chunk's QKV tiles are woven
    between the attention items so the in-order engine queues never
    head-of-line block; input DMAs are split across the SP/Act HWDGE queues
    in latency priority order

Note: fp8 (DoubleRow) variants of S/PV were measured at 3.5-5e-2 relative
error on this data - multiplicative fp8 operand noise does not average down
over keys for zero-mean Gaussian v - so the datapath stays bf16
(measured rel err 5.7e-3 vs the fp32 reference; sim estimate ~93us vs the
baseline's 110us sim / 124us HW).
"""

import numpy as np

B, T, C, H = 1, 2048, 1024, 16
D = C // H  # 64
ROPE_THETA = 10000.0
N_CORES = 8
MT = T // 128  # 16 m-tiles / j-tiles
NIC = T // 512  # 4 i-chunks

_CACHE = {}


def build_module():
    import concourse.bass as bass
    import concourse.mybir as mybir
    import concourse.tile as tile
    from concourse import bacc

    f32 = mybir.dt.float32
    bf16 = mybir.dt.bfloat16
    f8 = mybir.dt.float8e4
    EXP = mybir.ActivationFunctionType.Exp
    GE = mybir.AluOpType.is_ge
    DR = mybir.MatmulPerfMode.DoubleRow

    nc = bacc.Bacc("TRN2", target_bir_lowering=False, debug=False,
                   num_devices=N_CORES)

    xT_in = nc.declare_dram_parameter("xT_in", [C, T], bf16, isOutput=False)
    w_l = nc.declare_dram_parameter("w_l", [C, 3 * 128], bf16, isOutput=False)
    wp_l = nc.declare_dram_parameter("wp_l", [128, C], bf16, isOutput=False)
    cos_t = nc.declare_dram_parameter("cos_t", [T, 32], f32, isOutput=False)
    sin_t = nc.declare_dram_parameter("sin_t", [T, 32], f32, isOutput=False)
    out_p = nc.declare_dram_parameter("out_p", [T, C], bf16, isOutput=True)

    with tile.TileContext(nc) as tc:
        with tc.tile_pool(name="singles", bufs=1) as singles:
            w_sb = singles.tile([128, 8, 384], bf16)
            w_r = w_l.ap().rearrange("(kt p) n -> p kt n", p=128)
            wp_sb = singles.tile([128, 1024], bf16)
            ident = singles.tile([128, 128], bf16)
            ebias = singles.tile([128, 1], f32)
            nc.gpsimd.memset(ebias, -2.0)
            nc.gpsimd.memset(ident, 0.0)
            nc.gpsimd.affine_select(
                out=ident, in_=ident, compare_op=mybir.AluOpType.not_equal,
                fill=1.0, base=0, pattern=[[-1, 128]], channel_multiplier=1)

            # v in bf16: v_t[jt][:, 65h:65h+65] = [v | ones] per key-tile
            # (bf16 keeps PV noise out of the output: fp8 operand noise does
            # not average down over keys)
            v_t = [singles.tile([128, 130], bf16, name=f"v16_{jt}",
                                tag=f"v16_{jt}") for jt in range(MT)]
            for jt in range(MT):
                nc.vector.memset(v_t[jt][:, 64:65], 1.0)
                nc.vector.memset(v_t[jt][:, 129:130], 1.0)

            # bf16 rope'd q,k in (dim, T) layout per m-tile (tile-granular
            # deps: attention starts as soon as individual casts land);
            # partitions are [h0: perm'd dims (64), h1: perm'd dims (64)]
            qT_t = [singles.tile([128, 128], bf16, name=f"qTt{m}",
                                 tag=f"qTt{m}") for m in range(MT)]
            kT_t = [singles.tile([128, 128], bf16, name=f"kTt{m}",
                                 tag=f"kTt{m}") for m in range(MT)]

            # normalized attention (queries, dims) per chunk + transposed form
            at_sb = [singles.tile([128, 4, 128], bf16, name=f"at{ic}",
                                  tag=f"at{ic}") for ic in range(NIC)]
            aT_t = [singles.tile([128, 512], bf16, name=f"aT{ic}",
                                 tag=f"aT{ic}") for ic in range(NIC)]

            # x^T resident in SBUF per 512-col chunk (one DMA each, so the
            # first QKV tiles only wait on the first transfer)
            xT_sb = [singles.tile([128, 8, 512], bf16, name=f"xTs{c}",
                                  tag=f"xTs{c}") for c in range(NIC)]

            # xT chunk 0 on SP; everything else rides the Act queue in
            # priority order (w, cos, sin before the bulk xT tail) so the
            # serialized DMA engine services the latency-critical params
            # first. Dispatches are dependency-triggered, not program-order,
            # so queue membership is the only ordering lever.
            xT_r = xT_in.ap().rearrange("(kt p) t -> p kt t", p=128)
            nc.sync.dma_start(out=xT_sb[0][:, 0:4, :],
                              in_=xT_r[:, 0:4, 0:512])
            nc.scalar.dma_start(out=w_sb[:, 0:4, :], in_=w_r[:, 0:4, :])
            nc.sync.dma_start(out=xT_sb[0][:, 4:8, :],
                              in_=xT_r[:, 4:8, 0:512])
            nc.scalar.dma_start(out=w_sb[:, 4:8, :], in_=w_r[:, 4:8, :])
            cos_sb = singles.tile([128, MT, 32], f32)
            nc.scalar.dma_start(out=cos_sb, in_=cos_t.ap().rearrange(
                "(mt p) d -> p mt d", p=128))
            sin_sb = singles.tile([128, MT, 32], f32)
            nc.scalar.dma_start(out=sin_sb, in_=sin_t.ap().rearrange(
                "(mt p) d -> p mt d", p=128))
            nc.scalar.dma_start(out=wp_sb, in_=wp_l[:, :])
            for c in range(1, NIC):
                nc.scalar.dma_start(out=xT_sb[c],
                                    in_=xT_r[:, :, 512 * c:512 * c + 512])

            with \
                 tc.tile_pool(name="qkv_ps", bufs=3, space="PSUM") as qkvpool, \
                 tc.tile_pool(name="s_ps", bufs=3, space="PSUM") as spool, \
                 tc.tile_pool(name="o_ps", bufs=1, space="PSUM") as opool, \
                 tc.tile_pool(name="tmp", bufs=6) as tmppool, \
                 tc.tile_pool(name="rqk", bufs=6) as rqkpool, \
                 tc.tile_pool(name="e8p", bufs=14) as epool, \
                 tc.tile_pool(name="rt", bufs=4) as rpool, \
                 tc.tile_pool(name="osb", bufs=3) as ospool:

                rqk_live = {}

                def emit_qkv_mm(m):
                    qkv_ps = qkvpool.tile([128, 384], f32, name=f"qkv{m}",
                                          tag="qkv")
                    for k in range(8):
                        nc.tensor.matmul(
                            qkv_ps,
                            xT_sb[m // 4][:, k, 128 * (m % 4):128 * (m % 4) + 128],
                            w_sb[:, k, :],
                            start=(k == 0), stop=(k == 7))

                    cos_b = bass.AP(tensor=cos_sb.tensor,
                                    offset=cos_sb[:, m, :].offset,
                                    ap=[cos_sb.ap[0], [0, 8], [1, 32]])
                    sin_b = bass.AP(tensor=sin_sb.tensor,
                                    offset=sin_sb[:, m, :].offset,
                                    ap=[sin_sb.ap[0], [0, 8], [1, 32]])
                    src8 = qkv_ps[:, 0:256].rearrange("p (b d) -> p b d", b=8)
                    tcos = tmppool.tile([128, 8, 32], f32, name=f"tc_{m}",
                                        tag="tc")
                    tsin = tmppool.tile([128, 8, 32], f32, name=f"ts_{m}",
                                        tag="ts")
                    nc.vector.tensor_mul(tcos, src8, cos_b)
                    nc.vector.tensor_mul(tsin, src8, sin_b)
                    # col order per block: [q_h0(e|o), q_h1, k_h0, k_h1]
                    rqk = rqkpool.tile([128, 4, 64], bf16, name=f"rqk{m}",
                                       tag="rqk")
                    rqk_live[m] = rqk
                    tc4 = tcos.rearrange("p (b two) d -> p b two d", two=2)
                    ts4 = tsin.rearrange("p (b two) d -> p b two d", two=2)
                    nc.gpsimd.tensor_sub(rqk[:, :, 0:32],
                                         tc4[:, :, 0, :], ts4[:, :, 1, :])
                    nc.gpsimd.tensor_add(rqk[:, :, 32:64],
                                         ts4[:, :, 0, :], tc4[:, :, 1, :])

                    # v cast copies
                    nc.vector.tensor_copy(v_t[m][:, 0:64],
                                          qkv_ps[:, 256:320])
                    nc.vector.tensor_copy(v_t[m][:, 65:129],
                                          qkv_ps[:, 320:384])

                def emit_qkv_tp(m):
                    # transpose rope'd q,k to (dim, T) and cast to fp8
                    rqk = rqk_live.pop(m)
                    rqk2 = rqk.rearrange("p b d -> p (b d)")
                    tp_ps = qkvpool.tile([128, 256], bf16, name=f"tp{m}",
                                         tag="qkv")
                    nc.tensor.transpose(tp_ps[:, 0:128], rqk2[:, 0:128], ident)
                    nc.tensor.transpose(tp_ps[:, 128:256], rqk2[:, 128:256],
                                        ident)
                    nc.vector.tensor_copy(qT_t[m], tp_ps[:, 0:128])
                    nc.vector.tensor_copy(kT_t[m], tp_ps[:, 128:256])

                def qkv_steps(ms):
                    """Pipelined step list: mm(m+1) lands between mm(m) and
                    tp(m) so the transposes never head-of-line block PE."""
                    steps = []
                    for idx, m in enumerate(ms):
                        steps.append(("mm", m))
                        if idx >= 1:
                            steps.append(("tp", ms[idx - 1]))
                    if ms:
                        steps.append(("tp", ms[-1]))
                    return steps

                def run_step(st):
                    kind, m = st
                    (emit_qkv_mm if kind == "mm" else emit_qkv_tp)(m)

                def emit_S(ic, p2, e8s):
                    jt0 = 2 * p2
                    mm0 = jt0 - 4 * ic
                    for h in range(2):
                        e8_t = epool.tile([128, 4, 2, 128], bf16,
                                          name=f"e{ic}_{p2}_{h}", tag="e8")
                        e8s[(p2, h)] = e8_t
                        for i in range(2):
                            jt = jt0 + i
                            mm = mm0 + i
                            s_lo = max(0, mm)
                            s_t = spool.tile([128, 512], f32,
                                             name=f"s{ic}_{p2}_{h}_{i}",
                                             tag="s")
                            for s in range(s_lo, 4):
                                nc.tensor.matmul(
                                    s_t[:, 128 * s:128 * s + 128],
                                    kT_t[jt][64 * h:64 * h + 64, :],
                                    qT_t[4 * ic + s][64 * h:64 * h + 64, :],
                                    start=True, stop=True,
                                    skip_group_check=True)
                            nc.scalar.activation(
                                e8_t[:, s_lo:4, i, :],
                                s_t[:, 128 * s_lo:512].rearrange(
                                    "p (s q) -> p s q", q=128),
                                EXP, scale=0.125, bias=ebias[:, :])
                            if mm >= 0:
                                # only the 128-wide diagonal block needs the
                                # causal mask (columns beyond it always kept)
                                nc.gpsimd.affine_select(
                                    out=e8_t[:, mm, i, :],
                                    in_=e8_t[:, mm, i, :],
                                    compare_op=GE, fill=0.0, base=0,
                                    pattern=[[1, 128]],
                                    channel_multiplier=-1)

                def emit_PV(ic, p2, o_ps, J, e8s):
                    jt0 = 2 * p2
                    mm0 = jt0 - 4 * ic
                    for h in range(2):
                        e8_t = e8s.pop((p2, h))
                        for i in range(2):
                            s0 = max(0, mm0 + i)
                            for s in range(s0, 4):
                                nc.tensor.matmul(
                                    o_ps[h][:, s, 0:65],
                                    e8_t[:, s, i, :],
                                    v_t[jt0 + i][:, 65 * h:65 * h + 65],
                                    start=(p2 == 0 and s == 0 and i == 0),
                                    stop=(p2 == J // 2 - 1 and s == 3
                                          and i == 1),
                                    skip_group_check=True)

                def emit_norm_half(ic, sl, o_ps):
                    s0, s1 = 2 * sl, 2 * sl + 2
                    for h in range(2):
                        r_t = rpool.tile([128, 2, 1], f32,
                                         name=f"r{ic}_{sl}_{h}", tag="rt")
                        nc.vector.reciprocal(r_t, o_ps[h][:, s0:s1, 64:65])
                        r_b = bass.AP(tensor=r_t.tensor, offset=r_t.offset,
                                      ap=[r_t.ap[0], [1, 2], [0, 64]])
                        nc.vector.tensor_mul(
                            at_sb[ic][:, s0:s1, 64 * h:64 * h + 64],
                            o_ps[h][:, s0:s1, 0:64], r_b)
                    for s in (s0, s0 + 1):
                        tp2 = spool.tile([128, 512], bf16,
                                         name=f"atp{ic}_{s}", tag="s")
                        nc.tensor.transpose(tp2[:, 0:128],
                                            at_sb[ic][:, s, :], ident)
                        nc.vector.tensor_copy(
                            aT_t[ic][:, 128 * s:128 * s + 128],
                            tp2[:, 0:128])

                osb_live = {}

                def emit_proj_half(ic, sl):
                    if sl == 0:
                        osb_live[ic] = ospool.tile([128, 4, 1024], bf16,
                                                   name=f"os{ic}", tag="os")
                    o_sb = osb_live[ic]
                    for s in (2 * sl, 2 * sl + 1):
                        for n2 in range(2):
                            p_ps = spool.tile([128, 512], f32,
                                              name=f"p{ic}_{s}_{n2}", tag="s")
                            nc.tensor.matmul(
                                p_ps,
                                aT_t[ic][:, 128 * s:128 * s + 128],
                                wp_sb[:, 512 * n2:512 * n2 + 512],
                                start=True, stop=True, skip_group_check=True)
                            if n2 == 0:
                                nc.vector.tensor_copy(
                                    o_sb[:, s, 0:512], p_ps)
                            else:
                                nc.scalar.copy(
                                    o_sb[:, s, 512:1024], p_ps)
                    out_r = out_p.ap().rearrange("(n s p) c -> p n s c",
                                                 s=4, p=128)
                    if ic == NIC - 1:
                        for s in (2 * sl, 2 * sl + 1):
                            nc.sync.dma_start(out=out_r[:, ic, s, :],
                                              in_=o_sb[:, s, :])
                    else:
                        nc.sync.dma_start(
                            out=out_r[:, ic, 2 * sl:2 * sl + 2, :],
                            in_=o_sb[:, 2 * sl:2 * sl + 2, :])
                    if sl == 1:
                        del osb_live[ic]

                # ---- schedule ----
                for st in qkv_steps(list(range(4))):
                    run_step(st)
                o_live = {}
                for ic in range(NIC):
                    J = 4 * ic + 4
                    npairs = J // 2
                    o_ps = [opool.tile([128, 4, 128], f32, name=f"o{ic}_{h}",
                                       tag=f"o{h}") for h in range(2)]
                    o_live[ic] = o_ps
                    e8s = {}

                    # previous chunk's norm/proj ride early in this chunk's
                    # stream (norm halves before the first PV so the o_ps
                    # bank recycle never deadlocks the in-order queues)
                    from collections import deque
                    prevq = deque()
                    if ic - 1 in o_live and ic - 1 >= 0 and ic != 0:
                        prevq = deque([("normp", 0), ("normp", 1),
                                       ("projp", 0), ("projp", 1)])

                    seq = []
                    for p2 in range(npairs):
                        seq.append(("S", p2))
                        if prevq:
                            seq.append(prevq.popleft())
                        if p2 >= 1:
                            seq.append(("PV", p2 - 1))
                        if prevq:
                            seq.append(prevq.popleft())
                        if ic == NIC - 1 and p2 == npairs - 1:
                            # last chunk: own norm/proj halves inline
                            seq.append(("norm0_own", 0))
                            seq.append(("proj0_own", 0))
                    seq.append(("PV", npairs - 1))
                    seq.extend(prevq)
                    if ic == NIC - 1:
                        seq.append(("norm1_own", 1))
                        seq.append(("proj1_own", 1))

                    fillers = qkv_steps(list(range(4 * ic + 4, 4 * ic + 8))) \
                        if ic + 1 < NIC else []
                    fi = 0
                    for it, (kind, arg) in enumerate(seq):
                        if kind == "S":
                            emit_S(ic, arg, e8s)
                        elif kind == "PV":
                            emit_PV(ic, arg, o_ps, J, e8s)
                        elif kind == "normp":
                            emit_norm_half(ic - 1, arg, o_live[ic - 1])
                        elif kind == "projp":
                            emit_proj_half(ic - 1, arg)
                        elif kind in ("norm0_own", "norm1_own"):
                            emit_norm_half(ic, arg, o_ps)
                        else:
                            emit_proj_half(ic, arg)
                        # drain fillers over the first ~60% of the stream so
                        # the next chunk's fold can be issued early
                        while fi < len(fillers) and \
                                (fi + 1) * len(seq) * 6 <= \
                                (it + 1) * len(fillers) * 10:
                            run_step(fillers[fi])
                            fi += 1
                    while fi < len(fillers):
                        run_step(fillers[fi])
                        fi += 1

    nc.compile()
    return nc


def host_inputs(x, w_qkv, w_proj):
    """Build per-core input maps from the full inputs."""
    import ml_dtypes

    bf = ml_dtypes.bfloat16
    x2 = np.asarray(x, dtype=np.float32).reshape(T, C)
    xT = np.ascontiguousarray(x2.T).astype(bf)
    wq = np.asarray(w_qkv, dtype=np.float32)
    wp = np.asarray(w_proj, dtype=np.float32)

    inv_freq = 1.0 / (ROPE_THETA ** (np.arange(0, D, 2, dtype=np.float32) / D))
    ang = np.arange(T, dtype=np.float32)[:, None] * inv_freq[None, :]
    cos_t = np.cos(ang).astype(np.float32)   # (T, 32)
    sin_t = np.sin(ang).astype(np.float32)

    perm = np.concatenate([np.arange(0, D, 2), np.arange(1, D, 2)])
    in_maps = []
    for c in range(N_CORES):
        h0, h1 = 2 * c, 2 * c + 1
        cols = []
        for h in (h0, h1):      # q blocks, permuted evens|odds
            cols.append(wq[:, h * D:(h + 1) * D][:, perm])
        for h in (h0, h1):      # k blocks, permuted
            cols.append(wq[:, C + h * D:C + (h + 1) * D][:, perm])
        for h in (h0, h1):      # v blocks, natural
            cols.append(wq[:, 2 * C + h * D:2 * C + (h + 1) * D])
        w_l = np.ascontiguousarray(np.concatenate(cols, axis=1)).astype(bf)
        wp_l = np.ascontiguousarray(wp[128 * c:128 * c + 128, :]).astype(bf)
        in_maps.append({
            "xT_in": xT, "w_l": w_l, "wp_l": wp_l,
            "cos_t": cos_t, "sin_t": sin_t,
        })
    return in_maps


def kernel(x, w_qkv, w_proj):
    from concourse.bass_utils import run_bass_kernel_spmd

    if "nc" not in _CACHE:
        _CACHE["nc"] = build_module()
    nc = _CACHE["nc"]

    in_maps = host_inputs(x, w_qkv, w_proj)
    res = run_bass_kernel_spmd(nc, in_maps, list(range(N_CORES)))
    out = np.zeros((T, C), dtype=np.float32)
    for c in range(N_CORES):
        out += res.results[c]["out_p"].astype(np.float32)
    return out.reshape(B, T, C)
